# revision 2
# baseline (speedup 1.0000x reference)
"""Trainium2 kernel for nn_KernalAnsatz_65481071409588.

Problem: 23-qubit quantum-kernel fidelity |<psi_x|psi_y>|^2 where
psi_a = V(params) . (RY(a_0) x ... x RY(a_22)) |0...0>, with the SAME
variational unitary V(params) (two layers of per-qubit RX/RY/RZ rotations
and CNOT rings) applied to both encoded states.

Algebraic structure used by this kernel: the initial RY layer produces a
product state phi_a = prod_q (cos(a_q/2)|0> + sin(a_q/2)|1>), and everything
after it is one fixed unitary V identical for both circuits.  Since unitaries
preserve inner products, <psi_x|psi_y> = <V phi_x|V phi_y> = <phi_x|phi_y>
= prod_q cos((x_q - y_q)/2).  Therefore

    output = prod_{q=0}^{22} cos^2((x_q - y_q)/2)

exactly (verified against a complex128 full 2^23 statevector simulation of
the reference circuit: agreement to ~6e-15 relative).

Sharding: the 23 qubit angle pairs are split 3-per-core across the 8
NeuronCores (slot 24 padded with x=y=0, contributing a neutral factor 1).
Each core computes its partial product on device:
  d = x - y                       (vector)
  t = d/2 + pi/2                  (vector, so sin(t) = cos(d/2))
  u = t - 2*pi*round(t/(2*pi))    (vector; rounding via f32->i32->f32 casts,
                                   needed because the scalar-engine Sin table
                                   is only accurate on [-pi, pi])
  s = Sin(u)                      (scalar engine; sin(u) = sin(t) = cos(d/2))
  p = reduce_mult(s * s)          (vector)
and the host multiplies the 8 partials in float64.
"""

import sys

import numpy as np

for _p in ("/opt/trn_rl_repo", "/root/.axon_site/_ro/trn_rl_repo"):
    if _p not in sys.path:
        sys.path.append(_p)

import concourse.bass as bass
from concourse import mybir
from concourse.bass_utils import run_bass_kernel_spmd

N_QUBITS = 23
N_CORES = 8
QPC = 3  # qubits per core; 8 * 3 = 24 slots, the last one is neutral padding

F32 = mybir.dt.float32
I32 = mybir.dt.int32
PI = float(np.pi)
TWO_PI = float(2.0 * np.pi)
INV_FOUR_PI = float(1.0 / (4.0 * np.pi))

_NC_CACHE = None


def _build_nc():
    """Per-core SPMD program: partial = prod_j cos^2((x_j - y_j)/2), j=0..QPC-1."""
    nc = bass.Bass()
    xq = nc.declare_dram_parameter("xq", [QPC], F32, isOutput=False)
    yq = nc.declare_dram_parameter("yq", [QPC], F32, isOutput=False)
    out = nc.declare_dram_parameter("partial", [1], F32, isOutput=True)

    with (
        nc.sbuf_tensor("sx", [1, QPC], F32) as sx,
        nc.sbuf_tensor("sy", [1, QPC], F32) as sy,
        nc.sbuf_tensor("sd", [1, QPC], F32) as sd,
        nc.sbuf_tensor("st", [1, QPC], F32) as st,
        nc.sbuf_tensor("sv", [1, QPC], F32) as sv,
        nc.sbuf_tensor("ski", [1, QPC], I32) as ski,
        nc.sbuf_tensor("skf", [1, QPC], F32) as skf,
        nc.sbuf_tensor("sw", [1, QPC], F32) as sw,
        nc.sbuf_tensor("su", [1, QPC], F32) as su,
        nc.sbuf_tensor("ss", [1, QPC], F32) as ss,
        nc.sbuf_tensor("ssq", [1, QPC], F32) as ssq,
        nc.sbuf_tensor("sp", [1, 1], F32) as sp,
        nc.semaphore("dma_sem") as dma_sem,
        nc.semaphore("c_sem") as c_sem,
        nc.Block() as block,
    ):
        # NOTE: engines do NOT interlock same-engine read-after-write hazards
        # (deep pipelines) — back-to-back dependent vector ops read stale SBUF.
        # Every dependent pair is therefore serialized through c_sem.
        @block.sync
        def _(sync):
            sync.dma_start(out=sx[:, :], in_=xq[None, :]).then_inc(dma_sem, 16)
            sync.dma_start(out=sy[:, :], in_=yq[None, :]).then_inc(dma_sem, 16)
            sync.wait_ge(c_sem, 10)
            sync.dma_start(out=out[None, :], in_=sp[:, :]).then_inc(dma_sem, 16)
            sync.wait_ge(dma_sem, 48)

        @block.vector
        def _(vector):
            vector.wait_ge(dma_sem, 32)
            # d = x - y
            vector.tensor_sub(sd[:, :], sx[:, :], sy[:, :]).then_inc(c_sem, 1)
            vector.wait_ge(c_sem, 1)
            # t = 0.5*d + pi/2  (sin(t) = cos(d/2))
            vector.tensor_scalar(st[:, :], sd[:, :], 0.5, PI / 2,
                                 mybir.AluOpType.mult,
                                 mybir.AluOpType.add).then_inc(c_sem, 1)
            # v = t/(2*pi) = d/(4*pi) + 1/4
            vector.tensor_scalar(sv[:, :], sd[:, :], INV_FOUR_PI, 0.25,
                                 mybir.AluOpType.mult,
                                 mybir.AluOpType.add).then_inc(c_sem, 1)
            vector.wait_ge(c_sem, 3)
            # k = round(v) via f32 -> i32 -> f32 (cast rounds to nearest)
            vector.tensor_copy(ski[:, :], sv[:, :]).then_inc(c_sem, 1)
            vector.wait_ge(c_sem, 4)
            vector.tensor_copy(skf[:, :], ski[:, :]).then_inc(c_sem, 1)
            vector.wait_ge(c_sem, 5)
            # u = t - 2*pi*k  in [-pi, pi]
            vector.tensor_scalar_mul(sw[:, :], skf[:, :],
                                     -TWO_PI).then_inc(c_sem, 1)
            vector.wait_ge(c_sem, 6)
            vector.tensor_add(su[:, :], st[:, :], sw[:, :]).then_inc(c_sem, 1)
            # square and multiply the QPC factors together
            vector.wait_ge(c_sem, 8)  # scalar engine wrote ss
            vector.tensor_mul(ssq[:, :], ss[:, :], ss[:, :]).then_inc(c_sem, 1)
            vector.wait_ge(c_sem, 9)
            vector.tensor_reduce(sp[:, :1], ssq[:, :], op=mybir.AluOpType.mult,
                                 axis=mybir.AxisListType.X).then_inc(c_sem, 1)

        @block.scalar
        def _(scalar):
            # s = sin(u) = cos((x-y)/2)  (Sin table accurate on [-pi, pi])
            scalar.wait_ge(c_sem, 7)
            scalar.activation(ss[:, :], su[:, :],
                              mybir.ActivationFunctionType.Sin).then_inc(c_sem, 1)

    return nc


def kernel(x: np.ndarray, y: np.ndarray, params: np.ndarray) -> np.ndarray:
    global _NC_CACHE
    if _NC_CACHE is None:
        _NC_CACHE = _build_nc()
    nc = _NC_CACHE

    # Shard the 23 qubit-angle pairs 3 per core; slot 24 padded with zeros
    # (d = 0 -> cos^2 = 1, a neutral factor).
    xp = np.zeros(N_CORES * QPC, np.float32)
    yp = np.zeros(N_CORES * QPC, np.float32)
    xp[:N_QUBITS] = np.asarray(x, np.float32).reshape(-1)
    yp[:N_QUBITS] = np.asarray(y, np.float32).reshape(-1)
    in_maps = [
        {"xq": xp[QPC * i : QPC * (i + 1)], "yq": yp[QPC * i : QPC * (i + 1)]}
        for i in range(N_CORES)
    ]

    results = run_bass_kernel_spmd(nc, in_maps, list(range(N_CORES))).results

    # Gather: multiply the 8 per-core partial products.
    acc = np.float64(1.0)
    for i in range(N_CORES):
        acc *= np.float64(results[i]["partial"].reshape(-1)[0])
    return np.asarray(acc, dtype=np.float32)


# revision 4
# speedup vs baseline: 40310.8652x; 40310.8652x over previous
"""Trainium2 kernel for nn_KernalAnsatz_65481071409588.

Problem: 23-qubit quantum-kernel fidelity |<psi_x|psi_y>|^2 where
psi_a = V(params) . (RY(a_0) x ... x RY(a_22)) |0...0>, with the SAME
variational unitary V(params) (two layers of per-qubit RX/RY/RZ rotations
and CNOT rings) applied to both encoded states.

Algebraic structure used by this kernel: the initial RY layer produces a
product state phi_a = prod_q (cos(a_q/2)|0> + sin(a_q/2)|1>), and everything
after it is one fixed unitary V identical for both circuits.  Since unitaries
preserve inner products, <psi_x|psi_y> = <V phi_x|V phi_y> = <phi_x|phi_y>
= prod_q cos((x_q - y_q)/2).  Therefore

    output = prod_{q=0}^{22} cos^2((x_q - y_q)/2)

exactly, for every (x, y, params) — verified against a complex128 full 2^23
statevector simulation of the reference circuit (agreement ~6e-15 relative),
with the float32 reference itself ~7e-7 relative from the exact value.

Sharding: the 23 qubit angle pairs are split 3-per-core across the 8
NeuronCores (slot 24 padded with x=y=0, contributing a neutral factor 1).
Each core computes its partial product  r = prod_j cos((x_j - y_j)/2)
on device:
  d  = x - y                          (vector)
  t  = d/2 + pi/2                     (vector; sin(t) = cos(d/2))
  ki = i32(d/(4*pi) + 1/4)            (vector; = round(t/2pi), the i32 output
                                       cast rounds to nearest)
  u  = -2*pi*ki + t     in [-pi, pi]  (vector scalar_tensor_tensor, i32 input
                                       converts; needed because the
                                       scalar-engine Sin table is only
                                       accurate on [-pi, pi])
  s  = Sin(u)                         (scalar engine; sin(u) = cos(d/2))
  r  = reduce_mult(s)                 (vector)
and the host computes (prod_i r_i)^2 in float64.

Timing (TimelineSim cost model): 7.4 us per core, ~1.6 us above the
hard floor of this I/O contract (input DMA + output DMA round trips
alone cost 5.8 us; an empty program costs 1.0 us).

NOTE: engines do NOT interlock same-engine read-after-write hazards (deep
pipelines) — back-to-back dependent ops on one engine read stale SBUF.
Every dependent pair is serialized through c_sem.
"""

import sys

import numpy as np

for _p in ("/opt/trn_rl_repo", "/root/.axon_site/_ro/trn_rl_repo"):
    if _p not in sys.path:
        sys.path.append(_p)

import concourse.bass as bass
from concourse import mybir
from concourse.bass_utils import run_bass_kernel_spmd

N_QUBITS = 23
N_CORES = 8
QPC = 3  # qubits per core; 8 * 3 = 24 slots, the last one is neutral padding

F32 = mybir.dt.float32
I32 = mybir.dt.int32
PI = float(np.pi)
TWO_PI = float(2.0 * np.pi)
INV_FOUR_PI = float(1.0 / (4.0 * np.pi))

_NC_CACHE = None


def _build_nc():
    """Per-core SPMD program: partial = prod_j cos((x_j - y_j)/2), j=0..QPC-1."""
    A = mybir.AluOpType
    nc = bass.Bass()
    xyq = nc.declare_dram_parameter("xyq", [2 * QPC], F32, isOutput=False)
    out = nc.declare_dram_parameter("partial", [1], F32, isOutput=True)

    with (
        nc.sbuf_tensor("sxy", [1, 2 * QPC], F32) as sxy,
        nc.sbuf_tensor("sd", [1, QPC], F32) as sd,
        nc.sbuf_tensor("st", [1, QPC], F32) as st,
        nc.sbuf_tensor("ski", [1, QPC], I32) as ski,
        nc.sbuf_tensor("su", [1, QPC], F32) as su,
        nc.sbuf_tensor("ss", [1, QPC], F32) as ss,
        nc.sbuf_tensor("sp", [1, 1], F32) as sp,
        nc.semaphore("dma_sem") as dma_sem,
        nc.semaphore("c_sem") as c_sem,
        nc.Block() as block,
    ):
        sx = sxy[:, 0:QPC]
        sy = sxy[:, QPC : 2 * QPC]

        @block.sync
        def _(sync):
            sync.dma_start(out=sxy[:, :], in_=xyq[None, :]).then_inc(dma_sem, 16)
            sync.wait_ge(c_sem, 6)
            sync.dma_start(out=out[None, :], in_=sp[:, :]).then_inc(dma_sem, 16)
            sync.wait_ge(dma_sem, 32)

        @block.vector
        def _(vector):
            vector.wait_ge(dma_sem, 16)
            vector.tensor_sub(sd[:, :], sx, sy).then_inc(c_sem, 1)
            vector.wait_ge(c_sem, 1)
            vector.tensor_scalar(st[:, :], sd[:, :], 0.5, PI / 2,
                                 A.mult, A.add).then_inc(c_sem, 1)
            vector.tensor_scalar(ski[:, :], sd[:, :], INV_FOUR_PI, 0.25,
                                 A.mult, A.add).then_inc(c_sem, 1)
            vector.wait_ge(c_sem, 3)
            vector.scalar_tensor_tensor(su[:, :], ski[:, :], -TWO_PI, st[:, :],
                                        A.mult, A.add).then_inc(c_sem, 1)
            vector.wait_ge(c_sem, 5)  # scalar engine wrote ss
            vector.tensor_reduce(sp[:, :1], ss[:, :], op=A.mult,
                                 axis=mybir.AxisListType.X).then_inc(c_sem, 1)

        @block.scalar
        def _(scalar):
            scalar.wait_ge(c_sem, 4)
            scalar.activation(ss[:, :], su[:, :],
                              mybir.ActivationFunctionType.Sin).then_inc(c_sem, 1)

    return nc


def kernel(x: np.ndarray, y: np.ndarray, params: np.ndarray) -> np.ndarray:
    global _NC_CACHE
    if _NC_CACHE is None:
        _NC_CACHE = _build_nc()
    nc = _NC_CACHE

    # Shard the 23 qubit-angle pairs 3 per core; slot 24 padded with zeros
    # (d = 0 -> cos = 1, a neutral factor).
    xp = np.zeros(N_CORES * QPC, np.float32)
    yp = np.zeros(N_CORES * QPC, np.float32)
    xp[:N_QUBITS] = np.asarray(x, np.float32).reshape(-1)
    yp[:N_QUBITS] = np.asarray(y, np.float32).reshape(-1)
    in_maps = [
        {"xyq": np.concatenate([xp[QPC * i : QPC * (i + 1)],
                                yp[QPC * i : QPC * (i + 1)]])}
        for i in range(N_CORES)
    ]

    results = run_bass_kernel_spmd(nc, in_maps, list(range(N_CORES))).results

    # Gather: multiply the 8 per-core partial products of cos((x_q-y_q)/2),
    # then square for |<psi_x|psi_y>|^2.
    acc = np.float64(1.0)
    for i in range(N_CORES):
        acc *= np.float64(results[i]["partial"].reshape(-1)[0])
    return np.asarray(acc * acc, dtype=np.float32)
